# revision 30
# baseline (speedup 1.0000x reference)
"""Causal self-attention kernel for Trainium2, 8 NeuronCores.

Full inputs: x [4, 2048, 1024], W_attn [1024, 3072], W_proj [1024, 1024].
Sharding: 4-way data parallel over batch x 2-way tensor parallel over heads
(8 of 16 heads per group). Core c -> (batch b = c // 2, head-group g = c % 2).
Each core computes a partial output [2048, 1024] (its head-group's slice of
the attention output through its rows of W_proj); host sums the two partials
per batch.

Per-core kernel (T=2048, C=1024, 8 local heads, D=64, C_local=512):
  A) xT via PE transpose; qT/kT [512+512, 2048] and v-natural [2048, 512]
     (+ ones column per head for softmax row sums), all fp32r.
  B) per head, per k-tile of 128: scores sT = kT.T @ qT (PSUM) -> exp on ACT
     (scale 1/8, no max subtraction: scores are O(1) by construction) ->
     causal mask on the diagonal block via affine_select -> PV accumulation
     yT[65, q-chunk] += [v_h | 1].T @ exp(sT), softmax sums land in row 64.
     Normalize: broadcast 1/l over partitions via a PE outer product with a
     ones vector + DVE reciprocal + DVE multiply into yT_all.
  C) out = yT_all.T @ W_proj -> DMA out.
"""

import numpy as np

import concourse.bass as bass
import concourse.bacc as bacc
import concourse.mybir as mybir
import concourse.tile as tile
from concourse.bass_utils import run_bass_kernel_spmd
from concourse.masks import make_identity

F32 = mybir.dt.float32
F32R = mybir.dt.float32r

P = 128
T = 2048
C = 1024
HL = 8  # heads per core
D = 64
CL = HL * D  # 512
NKT = T // P  # 16 k tiles
NQC = 4  # q chunks of 512
QCH = 512
SCALE = 1.0 / 8.0  # 1/sqrt(D)
EXP = mybir.ActivationFunctionType.Exp


def build_kernel():
    nc = bacc.Bacc("TRN2")
    x = nc.dram_tensor("x", [T, C], F32, kind="ExternalInput")
    wa = nc.dram_tensor("wa", [C, 3 * CL], F32, kind="ExternalInput")
    wp = nc.dram_tensor("wp", [CL, C], F32, kind="ExternalInput")
    out = nc.dram_tensor("out", [T, C], F32, kind="ExternalOutput")
    # DRAM bounce for the softmax-sum rows: DRAM sources allow step-0
    # (broadcast) access patterns, SBUF sources don't.
    l_dram = nc.dram_tensor("l_scratch", [HL * NQC, QCH], F32)

    with tile.TileContext(nc) as tc:
        # ---- persistent tensors --------------------------------------
        # qkT_all[:, r, :]: row-groups r=0..3 -> q head-pairs, r=4..7 -> k
        # head-pairs; within a group, partitions 0:64 = even head's D dims,
        # 64:128 = odd head's.
        qkT_all, _free_qk = tc.tile([P, 8, T], F32R, name="qkT_all")
        # v_sb[:, kt, h, 0:64] = v rows for k-tile kt, head h; col 64 = ones.
        v_sb, _free_v = tc.tile([P, NKT, HL, D + 1], F32R, name="v_sb")
        ones_r, _free_ones = tc.tile([P, D], F32R, name="ones_r")
        ones_f, _free_onesf = tc.tile([P, P], F32, name="ones_f")
        identity, _free_id = tc.tile([P, P], F32, name="identity")
        # tril[k, q] = 1.0 if q >= k else 0.0 (keep lower-triangular in q-major)
        tril, _free_tril = tc.tile([P, P], F32, name="tril")

        make_identity(nc, identity)
        # memset doesn't support f32r: build f32 ones, copy-cast to f32r
        nc.vector.memset(ones_f, 1.0)
        nc.vector.tensor_copy(ones_r, ones_f[:, 0:D])
        # softmax-sum column: v_sb[:, kt, h, 64] = 1.0 for every (kt, h)
        v_view = v_sb.rearrange("p a b c -> p (a b) c")  # [128, 128, 65]
        nc.vector.tensor_copy(v_view[:, :, D : D + 1], ones_f.unsqueeze(2))
        nc.vector.memset(tril, 1.0)
        nc.gpsimd.affine_select(
            out=tril,
            in_=tril,
            compare_op=mybir.AluOpType.is_ge,
            fill=0.0,
            base=0,
            pattern=[[1, P]],
            channel_multiplier=-1,
        )

        # ---- phase A: xT, qkT, v -------------------------------------
        with (
            tc.tile_pool(name="wstage", bufs=2) as wstage_p,
            tc.tile_pool(name="wa_rp", bufs=1) as wa_rp,
            tc.tile_pool(name="xnat", bufs=2) as xnat_p,
            tc.tile_pool(name="xtp", bufs=2) as xtp_p,
            tc.tile_pool(name="ps_tp", bufs=2, space="PSUM") as ps_tp_p,
            tc.tile_pool(name="ps_qk", bufs=4, space="PSUM") as ps_qk_p,
            tc.tile_pool(name="ps_v", bufs=2, space="PSUM") as ps_v_p,
        ):
            wa_r = wa_rp.tile([P, 8, 3 * CL], F32R)
            wa_t = wa.rearrange("(c p) n -> p c n", p=P)  # [128, 8, 1536]
            for c in range(8):
                wst = wstage_p.tile([P, 3 * CL], F32, tag="wst")
                nc.sync.dma_start(out=wst, in_=wa_t[:, c, :])
                nc.scalar.copy(wa_r[:, c, :], wst)

            for tcp in range(2):  # pairs of 512-token chunks
                xT_pair = []
                for half in range(2):
                    tcn = tcp * 2 + half
                    xT_t = xtp_p.tile([P, 8, QCH], F32R, tag="xT", name=f"xT_{tcn}")
                    xT_pair.append(xT_t)
                    for tt in range(4):
                        xn = xnat_p.tile([P, C], F32, tag="xn", name=f"xn_{tcn}_{tt}")
                        t0 = tcn * QCH + tt * P
                        nc.sync.dma_start(out=xn, in_=x[t0 : t0 + P, :])
                        for c in range(8):
                            ps = ps_tp_p.tile([P, P], F32, tag="tp", name=f"tp_{tcn}_{tt}_{c}")
                            nc.tensor.transpose(ps, xn[:, c * P : (c + 1) * P], identity)
                            nc.vector.tensor_copy(
                                xT_t[:, c, tt * P : (tt + 1) * P], ps
                            )
                # qT / kT row groups; each W stationary feeds both chunks
                for r in range(8):
                    pss = [
                        ps_qk_p.tile([P, QCH], F32, tag="qk", name=f"qk_{tcp}_{r}_{i}")
                        for i in range(2)
                    ]
                    for c in range(8):
                        for half in range(2):
                            nc.tensor.matmul(
                                pss[half],
                                lhsT=wa_r[:, c, r * P : (r + 1) * P],
                                rhs=xT_pair[half][:, c, :],
                                start=(c == 0),
                                stop=(c == 7),
                            )
                    for half in range(2):
                        tcn = tcp * 2 + half
                        nc.vector.tensor_copy(
                            qkT_all[:, r, tcn * QCH : (tcn + 1) * QCH], pss[half]
                        )
                # v natural
                for half in range(2):
                    tcn = tcp * 2 + half
                    for tt in range(4):
                        ps = ps_v_p.tile([P, CL], F32, tag="v", name=f"v_{tcn}_{tt}")
                        for c in range(8):
                            nc.tensor.matmul(
                                ps,
                                lhsT=xT_pair[half][:, c, tt * P : (tt + 1) * P],
                                rhs=wa_r[:, c, 2 * CL : 3 * CL],
                                start=(c == 0),
                                stop=(c == 7),
                            )
                        kt = tcn * 4 + tt
                        nc.scalar.copy(
                            v_sb[:, kt, :, 0:D],
                            ps.rearrange("p (h d) -> p h d", h=HL),
                        )

        # ---- phase B: attention --------------------------------------
        # yT_all[:, p, t]: partition 64*(h%2)+d of pair p = normalized y^T.
        # Allocated after phase A closes so it reuses W_attn's SBUF space.
        yT_all, _free_yT = tc.tile([P, 4, T], F32R, name="yT_all")
        # K=128-padded q/k staging for the scores matmul: rows 0:64 hold the
        # current head's qT/kT, rows 64:128 stay zero. A 64-row contraction
        # would put the PE in row-tiled mode, which doesn't count as PE
        # activity for the HAM clock gate -> whole phase stuck at 1.2 GHz.
        # Two fixed buffers ping-pong across heads.
        qk_pads = []
        _pad_frees = []
        for pb in range(2):
            padt, fr = tc.tile([P, 2, T], F32R, name=f"qk_pad{pb}")
            qk_pads.append(padt)
            _pad_frees.append(fr)
        zf, _free_zf = tc.tile([D, T], F32, name="zf")
        nc.vector.memset(zf, 0.0)
        for pb in range(2):
            nc.vector.tensor_copy(qk_pads[pb][D:P, 0, :], zf)
            nc.vector.tensor_copy(qk_pads[pb][D:P, 1, :], zf)
        _free_zf()
        with (
            tc.tile_pool(name="aT", bufs=4) as aT_p,
            tc.tile_pool(name="raw", bufs=2) as raw_p,
            tc.tile_pool(name="recip", bufs=1) as recip_p,
            tc.tile_pool(name="ps_s", bufs=2, space="PSUM") as ps_s_p,
            tc.tile_pool(name="ps_y", bufs=4, space="PSUM") as ps_y_p,
        ):
            def emit_pad_copies(h):
                p_, e = h // 2, h % 2
                qoff = D * e
                pad = qk_pads[h % 2]
                nc.sync.dma_start(
                    out=pad[0:D, 0, :], in_=qkT_all[qoff : qoff + D, p_, :]
                )
                nc.sync.dma_start(
                    out=pad[0:D, 1, :], in_=qkT_all[qoff : qoff + D, 4 + p_, :]
                )

            emit_pad_copies(0)
            for h in range(HL):
                p_, e = h // 2, h % 2
                qoff = D * e
                pad = qk_pads[h % 2]
                qT_h = pad[:, 0, :]
                kT_h = pad[:, 1, :]

                yT_t = [
                    ps_y_p.tile([D + 1, QCH], F32, tag="yT", name=f"yT_{h}_{j}")
                    for j in range(NQC)
                ]
                aT_tiles = [None] * NKT

                def emit_scores(i):
                    span = T - P * i
                    aT_i = aT_p.tile([P, T], F32R, tag="aT")
                    aT_tiles[i] = aT_i
                    # high columns first: PV consumes descending-j, and the
                    # causal mask (cols 0:128) is only needed by the last
                    # (diagonal) PV chunk, so it leaves the critical path
                    subs = list(range(0, span, 1024))
                    for sub in reversed(subs):
                        sw = min(1024, span - sub)
                        ps = ps_s_p.tile([P, 1024], F32, tag="s")
                        for half in range(0, sw, 512):
                            ncol = min(512, sw - half)
                            nc.tensor.matmul(
                                ps[:, half : half + ncol],
                                lhsT=kT_h[:, P * i : P * (i + 1)],
                                rhs=qT_h[:, P * i + sub + half : P * i + sub + half + ncol],
                                start=True,
                                stop=True,
                            )
                        nc.scalar.activation(
                            aT_i[:, sub : sub + sw], ps[:, :sw], EXP, scale=SCALE
                        )
                    # causal mask on the diagonal 128 columns: keep q >= k
                    nc.gpsimd.tensor_mul(aT_i[:, 0:P], aT_i[:, 0:P], tril)

                def emit_pv(i):
                    aT_i = aT_tiles[i]
                    jmin = (P * i) // QCH
                    for j in range(NQC - 1, jmin - 1, -1):
                        qs = QCH * j
                        out_off = max(P * i - qs, 0)
                        rel = max(qs - P * i, 0)
                        ncol = QCH - out_off
                        nc.tensor.matmul(
                            yT_t[j][:, out_off : out_off + ncol],
                            lhsT=v_sb[:, i, h, :],
                            rhs=aT_i[:, rel : rel + ncol],
                            start=(i == 0),
                            stop=(i == 4 * j + 3),
                        )
                    if i % 4 == 3:
                        emit_norm(i // 4)

                def emit_norm(j):
                    raw = raw_p.tile([D + 1, QCH], F32R, tag="raw")
                    nc.vector.tensor_copy(raw, yT_t[j])
                    # broadcast the softmax sums (row 64) across 64 partitions
                    # via a DRAM bounce (DMA with step-0 partition source);
                    # keeps the PE out of it (no mode switch, no PSUM bank).
                    idx = h * NQC + j
                    nc.sync.dma_start(
                        out=l_dram[idx, :], in_=raw[D : D + 1, :].bitcast(F32)
                    )
                    lrow = l_dram[idx, :]
                    bcast_src = bass.AP(
                        tensor=lrow.tensor,
                        offset=lrow.offset,
                        ap=[[0, D]] + [list(pair) for pair in lrow.ap],
                    )
                    bcast = recip_p.tile([D, QCH], F32, tag="bcast")
                    nc.sync.dma_start(out=bcast, in_=bcast_src)
                    recip = recip_p.tile([D, QCH], F32, tag="recip")
                    nc.vector.reciprocal_approx_fast(out=recip, in_=bcast)
                    nc.vector.tensor_mul(
                        yT_all[qoff : qoff + D, p_, QCH * j : QCH * (j + 1)],
                        raw[0:D, :],
                        recip,
                    )

                emit_scores(0)
                emit_scores(1)
                if h + 1 < HL:
                    # stage next head's padded q/k while this head computes
                    emit_pad_copies(h + 1)
                emit_pv(0)
                for i in range(2, NKT):
                    emit_scores(i)
                    emit_pv(i - 1)
                emit_pv(NKT - 1)

        # ---- phase C: output projection ------------------------------
        with (
            tc.tile_pool(name="wpstage", bufs=2) as wpstage_p,
            tc.tile_pool(name="wp_rp", bufs=1) as wp_rp,
            tc.tile_pool(name="outp", bufs=3) as out_p,
            tc.tile_pool(name="ps_o", bufs=4, space="PSUM") as ps_o_p,
        ):
            wp_r = wp_rp.tile([P, 4, C], F32R)
            wp_t = wp.rearrange("(c p) n -> p c n", p=P)  # [128, 4, 1024]
            for cc in range(4):
                wst = wpstage_p.tile([P, C], F32, tag="wpst")
                nc.sync.dma_start(out=wst, in_=wp_t[:, cc, :])
                nc.scalar.copy(wp_r[:, cc, :], wst)

            for t in range(NKT):
                out_t = out_p.tile([P, C], F32, tag="out")
                ps_h = [
                    ps_o_p.tile([P, 512], F32, tag="o", name=f"ps_o_{t}_{hf}")
                    for hf in range(2)
                ]
                for p2 in range(4):
                    for half in range(2):
                        nc.tensor.matmul(
                            ps_h[half],
                            lhsT=yT_all[:, p2, P * t : P * (t + 1)],
                            rhs=wp_r[:, p2, 512 * half : 512 * (half + 1)],
                            start=(p2 == 0),
                            stop=(p2 == 3),
                        )
                for half in range(2):
                    nc.vector.tensor_copy(
                        out_t[:, 512 * half : 512 * (half + 1)], ps_h[half]
                    )
                nc.sync.dma_start(out=out[P * t : P * (t + 1), :], in_=out_t)

    nc.finalize()
    return nc


_NC_CACHE = None


def kernel(x, W_attn, W_proj):
    global _NC_CACHE
    if _NC_CACHE is None:
        _NC_CACHE = build_kernel()
    nc = _NC_CACHE

    B = x.shape[0]
    H_TOTAL = 16
    x = np.asarray(x, dtype=np.float32)
    W_attn = np.asarray(W_attn, dtype=np.float32)
    W_proj = np.asarray(W_proj, dtype=np.float32)

    in_maps = []
    for core in range(8):
        b, g = core // 2, core % 2
        cols = slice(g * CL, (g + 1) * CL)
        wa_local = np.ascontiguousarray(
            np.concatenate(
                [
                    W_attn[:, 0 * C :][:, cols],
                    W_attn[:, 1 * C :][:, cols],
                    W_attn[:, 2 * C :][:, cols],
                ],
                axis=1,
            )
        )
        wp_local = np.ascontiguousarray(W_proj[g * CL : (g + 1) * CL, :])
        in_maps.append(
            {
                "x": np.ascontiguousarray(x[b]),
                "wa": wa_local,
                "wp": wp_local,
            }
        )

    res = run_bass_kernel_spmd(nc, in_maps, core_ids=list(range(8)))
    outs = [r["out"] for r in res.results]
    full = np.stack([outs[2 * b] + outs[2 * b + 1] for b in range(B)], axis=0)
    return full


# revision 31
# speedup vs baseline: 1.1876x; 1.1876x over previous
"""Causal self-attention kernel for Trainium2, 8 NeuronCores.

Full inputs: x [4, 2048, 1024], W_attn [1024, 3072], W_proj [1024, 1024].
Sharding: 4-way data parallel over batch x 2-way tensor parallel over heads
(8 of 16 heads per group). Core c -> (batch b = c // 2, head-group g = c % 2).
Each core computes a partial output [2048, 1024] (its head-group's slice of
the attention output through its rows of W_proj); host sums the two partials
per batch.

Per-core kernel (T=2048, C=1024, 8 local heads, D=64, C_local=512):
  A) xT via PE transpose; qT/kT [512+512, 2048] and v-natural [2048, 512]
     (+ ones column per head for softmax row sums), all fp32r.
  B) per head, per k-tile of 128: scores sT = kT.T @ qT (PSUM) -> exp on ACT
     (scale 1/8, no max subtraction: scores are O(1) by construction) ->
     causal mask on the diagonal block via affine_select -> PV accumulation
     yT[65, q-chunk] += [v_h | 1].T @ exp(sT), softmax sums land in row 64.
     Normalize: broadcast 1/l over partitions via a PE outer product with a
     ones vector + DVE reciprocal + DVE multiply into yT_all.
  C) out = yT_all.T @ W_proj -> DMA out.
"""

import numpy as np

import concourse.bass as bass
import concourse.bacc as bacc
import concourse.mybir as mybir
import concourse.tile as tile
from concourse.bass_utils import run_bass_kernel_spmd
from concourse.masks import make_identity

F32 = mybir.dt.float32
F32R = mybir.dt.float32r

P = 128
T = 2048
C = 1024
HL = 8  # heads per core
D = 64
CL = HL * D  # 512
NKT = T // P  # 16 k tiles
NQC = 4  # q chunks of 512
QCH = 512
SCALE = 1.0 / 8.0  # 1/sqrt(D)
EXP = mybir.ActivationFunctionType.Exp


def build_kernel():
    nc = bacc.Bacc("TRN2")
    x = nc.dram_tensor("x", [T, C], F32, kind="ExternalInput")
    wa = nc.dram_tensor("wa", [C, 3 * CL], F32, kind="ExternalInput")
    wp = nc.dram_tensor("wp", [CL, C], F32, kind="ExternalInput")
    out = nc.dram_tensor("out", [T, C], F32, kind="ExternalOutput")
    # DRAM bounce for the softmax-sum rows: DRAM sources allow step-0
    # (broadcast) access patterns, SBUF sources don't.
    l_dram = nc.dram_tensor("l_scratch", [HL * NQC, QCH], F32)

    with tile.TileContext(nc) as tc:
        # ---- persistent tensors --------------------------------------
        # qkT_all[:, r, :]: row-groups r=0..3 -> q head-pairs, r=4..7 -> k
        # head-pairs; within a group, partitions 0:64 = even head's D dims,
        # 64:128 = odd head's.
        qkT_all, _free_qk = tc.tile([P, 8, T], F32R, name="qkT_all")
        # v_sb[:, kt, h, 0:64] = v rows for k-tile kt, head h; col 64 = ones.
        v_sb, _free_v = tc.tile([P, NKT, HL, D + 1], F32R, name="v_sb")
        ones_r, _free_ones = tc.tile([P, D], F32R, name="ones_r")
        ones_f, _free_onesf = tc.tile([P, P], F32, name="ones_f")
        identity, _free_id = tc.tile([P, P], F32, name="identity")
        # tril[k, q] = 1.0 if q >= k else 0.0 (keep lower-triangular in q-major)
        tril, _free_tril = tc.tile([P, P], F32, name="tril")

        make_identity(nc, identity)
        # memset doesn't support f32r: build f32 ones, copy-cast to f32r
        nc.vector.memset(ones_f, 1.0)
        nc.vector.tensor_copy(ones_r, ones_f[:, 0:D])
        # softmax-sum column: v_sb[:, kt, h, 64] = 1.0 for every (kt, h)
        v_view = v_sb.rearrange("p a b c -> p (a b) c")  # [128, 128, 65]
        nc.vector.tensor_copy(v_view[:, :, D : D + 1], ones_f.unsqueeze(2))
        nc.vector.memset(tril, 1.0)
        nc.gpsimd.affine_select(
            out=tril,
            in_=tril,
            compare_op=mybir.AluOpType.is_ge,
            fill=0.0,
            base=0,
            pattern=[[1, P]],
            channel_multiplier=-1,
        )

        # ---- phase A: xT, qkT, v -------------------------------------
        with (
            tc.tile_pool(name="wstage", bufs=2) as wstage_p,
            tc.tile_pool(name="wa_rp", bufs=1) as wa_rp,
            tc.tile_pool(name="xnat", bufs=2) as xnat_p,
            tc.tile_pool(name="xtp", bufs=2) as xtp_p,
            tc.tile_pool(name="ps_tp", bufs=2, space="PSUM") as ps_tp_p,
            tc.tile_pool(name="ps_qk", bufs=4, space="PSUM") as ps_qk_p,
            tc.tile_pool(name="ps_v", bufs=2, space="PSUM") as ps_v_p,
        ):
            wa_r = wa_rp.tile([P, 8, 3 * CL], F32R)
            wa_t = wa.rearrange("(c p) n -> p c n", p=P)  # [128, 8, 1536]
            for c in range(8):
                wst = wstage_p.tile([P, 3 * CL], F32, tag="wst")
                nc.sync.dma_start(out=wst, in_=wa_t[:, c, :])
                nc.scalar.copy(wa_r[:, c, :], wst)

            for tcp in range(2):  # pairs of 512-token chunks
                xT_pair = []
                for half in range(2):
                    tcn = tcp * 2 + half
                    xT_t = xtp_p.tile([P, 8, QCH], F32R, tag="xT", name=f"xT_{tcn}")
                    xT_pair.append(xT_t)
                    for tt in range(4):
                        xn = xnat_p.tile([P, C], F32, tag="xn", name=f"xn_{tcn}_{tt}")
                        t0 = tcn * QCH + tt * P
                        nc.sync.dma_start(out=xn, in_=x[t0 : t0 + P, :])
                        for c in range(8):
                            ps = ps_tp_p.tile([P, P], F32, tag="tp", name=f"tp_{tcn}_{tt}_{c}")
                            nc.tensor.transpose(ps, xn[:, c * P : (c + 1) * P], identity)
                            nc.vector.tensor_copy(
                                xT_t[:, c, tt * P : (tt + 1) * P], ps
                            )
                # qT / kT row groups; each W stationary feeds both chunks
                for r in range(8):
                    pss = [
                        ps_qk_p.tile([P, QCH], F32, tag="qk", name=f"qk_{tcp}_{r}_{i}")
                        for i in range(2)
                    ]
                    for c in range(8):
                        for half in range(2):
                            nc.tensor.matmul(
                                pss[half],
                                lhsT=wa_r[:, c, r * P : (r + 1) * P],
                                rhs=xT_pair[half][:, c, :],
                                start=(c == 0),
                                stop=(c == 7),
                            )
                    for half in range(2):
                        tcn = tcp * 2 + half
                        nc.vector.tensor_copy(
                            qkT_all[:, r, tcn * QCH : (tcn + 1) * QCH], pss[half]
                        )
                # v natural
                for half in range(2):
                    tcn = tcp * 2 + half
                    for tt in range(4):
                        ps = ps_v_p.tile([P, CL], F32, tag="v", name=f"v_{tcn}_{tt}")
                        for c in range(8):
                            nc.tensor.matmul(
                                ps,
                                lhsT=xT_pair[half][:, c, tt * P : (tt + 1) * P],
                                rhs=wa_r[:, c, 2 * CL : 3 * CL],
                                start=(c == 0),
                                stop=(c == 7),
                            )
                        kt = tcn * 4 + tt
                        nc.scalar.copy(
                            v_sb[:, kt, :, 0:D],
                            ps.rearrange("p (h d) -> p h d", h=HL),
                        )

        # ---- phase B: attention --------------------------------------
        # yT_all[:, p, t]: partition 64*(h%2)+d of pair p = normalized y^T.
        # Allocated after phase A closes so it reuses W_attn's SBUF space.
        yT_all, _free_yT = tc.tile([P, 4, T], F32R, name="yT_all")
        # K=128-padded q/k staging for the scores matmul: rows 0:64 hold the
        # current head's qT/kT, rows 64:128 stay zero. A 64-row contraction
        # would put the PE in row-tiled mode, which doesn't count as PE
        # activity for the HAM clock gate -> whole phase stuck at 1.2 GHz.
        # Two fixed buffers ping-pong across heads.
        qk_pads = []
        _pad_frees = []
        for pb in range(2):
            padt, fr = tc.tile([P, 2, T], F32R, name=f"qk_pad{pb}")
            qk_pads.append(padt)
            _pad_frees.append(fr)
        zf, _free_zf = tc.tile([D, T], F32, name="zf")
        nc.vector.memset(zf, 0.0)
        for pb in range(2):
            nc.vector.tensor_copy(qk_pads[pb][D:P, 0, :], zf)
            nc.vector.tensor_copy(qk_pads[pb][D:P, 1, :], zf)
        _free_zf()
        with (
            tc.tile_pool(name="aT", bufs=3) as aT_p,
            tc.tile_pool(name="raw", bufs=3) as raw_p,
            tc.tile_pool(name="recip", bufs=2) as recip_p,
            tc.tile_pool(name="ps_s", bufs=2, space="PSUM") as ps_s_p,
            tc.tile_pool(name="ps_y", bufs=4, space="PSUM") as ps_y_p,
        ):
            def emit_pad_copies(h):
                p_, e = h // 2, h % 2
                qoff = D * e
                pad = qk_pads[h % 2]
                nc.sync.dma_start(
                    out=pad[0:D, 0, :], in_=qkT_all[qoff : qoff + D, p_, :]
                )
                nc.sync.dma_start(
                    out=pad[0:D, 1, :], in_=qkT_all[qoff : qoff + D, 4 + p_, :]
                )

            emit_pad_copies(0)
            for h in range(HL):
                p_, e = h // 2, h % 2
                qoff = D * e
                pad = qk_pads[h % 2]
                qT_h = pad[:, 0, :]
                kT_h = pad[:, 1, :]

                yT_t = [
                    ps_y_p.tile([D + 1, QCH], F32, tag="yT", name=f"yT_{h}_{j}")
                    for j in range(NQC)
                ]
                aT_tiles = [None] * NKT

                def emit_scores(i):
                    span = T - P * i
                    aT_i = aT_p.tile([P, T], F32R, tag="aT")
                    aT_tiles[i] = aT_i
                    # high columns first: PV consumes descending-j, and the
                    # causal mask (cols 0:128) is only needed by the last
                    # (diagonal) PV chunk, so it leaves the critical path
                    subs = list(range(0, span, 1024))
                    for sub in reversed(subs):
                        sw = min(1024, span - sub)
                        ps = ps_s_p.tile([P, 1024], F32, tag="s")
                        for half in range(0, sw, 512):
                            ncol = min(512, sw - half)
                            nc.tensor.matmul(
                                ps[:, half : half + ncol],
                                lhsT=kT_h[:, P * i : P * (i + 1)],
                                rhs=qT_h[:, P * i + sub + half : P * i + sub + half + ncol],
                                start=True,
                                stop=True,
                            )
                        nc.scalar.activation(
                            aT_i[:, sub : sub + sw], ps[:, :sw], EXP, scale=SCALE
                        )
                    # causal mask on the diagonal 128 columns: keep q >= k
                    nc.gpsimd.tensor_mul(aT_i[:, 0:P], aT_i[:, 0:P], tril)

                def emit_pv(i):
                    aT_i = aT_tiles[i]
                    jmin = (P * i) // QCH
                    for j in range(NQC - 1, jmin - 1, -1):
                        qs = QCH * j
                        out_off = max(P * i - qs, 0)
                        rel = max(qs - P * i, 0)
                        ncol = QCH - out_off
                        nc.tensor.matmul(
                            yT_t[j][:, out_off : out_off + ncol],
                            lhsT=v_sb[:, i, h, :],
                            rhs=aT_i[:, rel : rel + ncol],
                            start=(i == 0),
                            stop=(i == 4 * j + 3),
                        )
                    if i % 4 == 3:
                        emit_norm(i // 4)

                def emit_norm(j):
                    raw = raw_p.tile([D + 1, QCH], F32R, tag="raw")
                    nc.vector.tensor_copy(raw, yT_t[j])
                    # broadcast the softmax sums (row 64) across 64 partitions
                    # via a DRAM bounce (DMA with step-0 partition source);
                    # keeps the PE out of it (no mode switch, no PSUM bank).
                    idx = h * NQC + j
                    nc.sync.dma_start(
                        out=l_dram[idx, :], in_=raw[D : D + 1, :].bitcast(F32)
                    )
                    lrow = l_dram[idx, :]
                    bcast_src = bass.AP(
                        tensor=lrow.tensor,
                        offset=lrow.offset,
                        ap=[[0, D]] + [list(pair) for pair in lrow.ap],
                    )
                    bcast = recip_p.tile([D, QCH], F32, tag="bcast")
                    nc.sync.dma_start(out=bcast, in_=bcast_src)
                    recip = recip_p.tile([D, QCH], F32, tag="recip")
                    nc.vector.reciprocal_approx_fast(out=recip, in_=bcast)
                    nc.vector.tensor_mul(
                        yT_all[qoff : qoff + D, p_, QCH * j : QCH * (j + 1)],
                        raw[0:D, :],
                        recip,
                    )

                emit_scores(0)
                emit_scores(1)
                if h + 1 < HL:
                    # stage next head's padded q/k while this head computes
                    emit_pad_copies(h + 1)
                emit_pv(0)
                for i in range(2, NKT):
                    emit_scores(i)
                    emit_pv(i - 1)
                emit_pv(NKT - 1)

        # ---- phase C: output projection ------------------------------
        with (
            tc.tile_pool(name="wpstage", bufs=2) as wpstage_p,
            tc.tile_pool(name="wp_rp", bufs=1) as wp_rp,
            tc.tile_pool(name="outp", bufs=3) as out_p,
            tc.tile_pool(name="ps_o", bufs=4, space="PSUM") as ps_o_p,
        ):
            wp_r = wp_rp.tile([P, 4, C], F32R)
            wp_t = wp.rearrange("(c p) n -> p c n", p=P)  # [128, 4, 1024]
            for cc in range(4):
                wst = wpstage_p.tile([P, C], F32, tag="wpst")
                nc.sync.dma_start(out=wst, in_=wp_t[:, cc, :])
                nc.scalar.copy(wp_r[:, cc, :], wst)

            for t in range(NKT):
                out_t = out_p.tile([P, C], F32, tag="out")
                ps_h = [
                    ps_o_p.tile([P, 512], F32, tag="o", name=f"ps_o_{t}_{hf}")
                    for hf in range(2)
                ]
                for p2 in range(4):
                    for half in range(2):
                        nc.tensor.matmul(
                            ps_h[half],
                            lhsT=yT_all[:, p2, P * t : P * (t + 1)],
                            rhs=wp_r[:, p2, 512 * half : 512 * (half + 1)],
                            start=(p2 == 0),
                            stop=(p2 == 3),
                        )
                for half in range(2):
                    nc.vector.tensor_copy(
                        out_t[:, 512 * half : 512 * (half + 1)], ps_h[half]
                    )
                nc.sync.dma_start(out=out[P * t : P * (t + 1), :], in_=out_t)

    nc.finalize()
    return nc


_NC_CACHE = None


def kernel(x, W_attn, W_proj):
    global _NC_CACHE
    if _NC_CACHE is None:
        _NC_CACHE = build_kernel()
    nc = _NC_CACHE

    B = x.shape[0]
    H_TOTAL = 16
    x = np.asarray(x, dtype=np.float32)
    W_attn = np.asarray(W_attn, dtype=np.float32)
    W_proj = np.asarray(W_proj, dtype=np.float32)

    in_maps = []
    for core in range(8):
        b, g = core // 2, core % 2
        cols = slice(g * CL, (g + 1) * CL)
        wa_local = np.ascontiguousarray(
            np.concatenate(
                [
                    W_attn[:, 0 * C :][:, cols],
                    W_attn[:, 1 * C :][:, cols],
                    W_attn[:, 2 * C :][:, cols],
                ],
                axis=1,
            )
        )
        wp_local = np.ascontiguousarray(W_proj[g * CL : (g + 1) * CL, :])
        in_maps.append(
            {
                "x": np.ascontiguousarray(x[b]),
                "wa": wa_local,
                "wp": wp_local,
            }
        )

    res = run_bass_kernel_spmd(nc, in_maps, core_ids=list(range(8)))
    outs = [r["out"] for r in res.results]
    full = np.stack([outs[2 * b] + outs[2 * b + 1] for b in range(B)], axis=0)
    return full


# revision 32
# speedup vs baseline: 1.2322x; 1.0375x over previous
"""Causal self-attention kernel for Trainium2, 8 NeuronCores.

Full inputs: x [4, 2048, 1024], W_attn [1024, 3072], W_proj [1024, 1024].
Sharding: 4-way data parallel over batch x 2-way tensor parallel over heads
(8 of 16 heads per group). Core c -> (batch b = c // 2, head-group g = c % 2).
Each core computes a partial output [2048, 1024] (its head-group's slice of
the attention output through its rows of W_proj); host sums the two partials
per batch.

Per-core kernel (T=2048, C=1024, 8 local heads, D=64, C_local=512):
  A) xT via PE transpose; qT/kT [512+512, 2048] and v-natural [2048, 512]
     (+ ones column per head for softmax row sums), all fp32r.
  B) per head, per k-tile of 128: scores sT = kT.T @ qT (PSUM) -> exp on ACT
     (scale 1/8, no max subtraction: scores are O(1) by construction) ->
     causal mask on the diagonal block via affine_select -> PV accumulation
     yT[65, q-chunk] += [v_h | 1].T @ exp(sT), softmax sums land in row 64.
     Normalize: broadcast 1/l over partitions via a PE outer product with a
     ones vector + DVE reciprocal + DVE multiply into yT_all.
  C) out = yT_all.T @ W_proj -> DMA out.
"""

import numpy as np

import concourse.bass as bass
import concourse.bacc as bacc
import concourse.mybir as mybir
import concourse.tile as tile
from concourse.bass_utils import run_bass_kernel_spmd
from concourse.masks import make_identity

F32 = mybir.dt.float32
F32R = mybir.dt.float32r

P = 128
T = 2048
C = 1024
HL = 8  # heads per core
D = 64
CL = HL * D  # 512
NKT = T // P  # 16 k tiles
NQC = 4  # q chunks of 512
QCH = 512
SCALE = 1.0 / 8.0  # 1/sqrt(D)
EXP = mybir.ActivationFunctionType.Exp


def build_kernel():
    nc = bacc.Bacc("TRN2")
    x = nc.dram_tensor("x", [T, C], F32, kind="ExternalInput")
    wa = nc.dram_tensor("wa", [C, 3 * CL], F32, kind="ExternalInput")
    wp = nc.dram_tensor("wp", [CL, C], F32, kind="ExternalInput")
    out = nc.dram_tensor("out", [T, C], F32, kind="ExternalOutput")
    # DRAM bounce for the softmax-sum rows: DRAM sources allow step-0
    # (broadcast) access patterns, SBUF sources don't.
    l_dram = nc.dram_tensor("l_scratch", [HL * NQC, QCH], F32)

    with tile.TileContext(nc) as tc:
        # ---- persistent tensors --------------------------------------
        # qkT_all[:, r, :]: row-groups r=0..3 -> q head-pairs, r=4..7 -> k
        # head-pairs; within a group, partitions 0:64 = even head's D dims,
        # 64:128 = odd head's.
        qkT_all, _free_qk = tc.tile([P, 8, T], F32R, name="qkT_all")
        # v_sb[:, kt, h, 0:64] = v rows for k-tile kt, head h; col 64 = ones.
        v_sb, _free_v = tc.tile([P, NKT, HL, D + 1], F32R, name="v_sb")
        ones_r, _free_ones = tc.tile([P, D], F32R, name="ones_r")
        ones_f, _free_onesf = tc.tile([P, P], F32, name="ones_f")
        identity, _free_id = tc.tile([P, P], F32, name="identity")
        # tril[k, q] = 1.0 if q >= k else 0.0 (keep lower-triangular in q-major)
        tril, _free_tril = tc.tile([P, P], F32, name="tril")

        make_identity(nc, identity)
        # memset doesn't support f32r: build f32 ones, copy-cast to f32r
        nc.vector.memset(ones_f, 1.0)
        nc.vector.tensor_copy(ones_r, ones_f[:, 0:D])
        # softmax-sum column: v_sb[:, kt, h, 64] = 1.0 for every (kt, h)
        v_view = v_sb.rearrange("p a b c -> p (a b) c")  # [128, 128, 65]
        nc.vector.tensor_copy(v_view[:, :, D : D + 1], ones_f.unsqueeze(2))
        nc.vector.memset(tril, 1.0)
        nc.gpsimd.affine_select(
            out=tril,
            in_=tril,
            compare_op=mybir.AluOpType.is_ge,
            fill=0.0,
            base=0,
            pattern=[[1, P]],
            channel_multiplier=-1,
        )

        # ---- phase A: xT, qkT, v -------------------------------------
        with (
            tc.tile_pool(name="wstage", bufs=2) as wstage_p,
            tc.tile_pool(name="wa_rp", bufs=1) as wa_rp,
            tc.tile_pool(name="xnat", bufs=2) as xnat_p,
            tc.tile_pool(name="xtp", bufs=2) as xtp_p,
            tc.tile_pool(name="ps_tp", bufs=2, space="PSUM") as ps_tp_p,
            tc.tile_pool(name="ps_qk", bufs=4, space="PSUM") as ps_qk_p,
            tc.tile_pool(name="ps_v", bufs=2, space="PSUM") as ps_v_p,
        ):
            wa_r = wa_rp.tile([P, 8, 3 * CL], F32R)
            wa_t = wa.rearrange("(c p) n -> p c n", p=P)  # [128, 8, 1536]
            for c in range(8):
                wst = wstage_p.tile([P, 3 * CL], F32, tag="wst")
                nc.sync.dma_start(out=wst, in_=wa_t[:, c, :])
                nc.scalar.copy(wa_r[:, c, :], wst)

            for tcp in range(2):  # pairs of 512-token chunks
                xT_pair = []
                for half in range(2):
                    tcn = tcp * 2 + half
                    xT_t = xtp_p.tile([P, 8, QCH], F32R, tag="xT", name=f"xT_{tcn}")
                    xT_pair.append(xT_t)
                    for tt in range(4):
                        xn = xnat_p.tile([P, C], F32, tag="xn", name=f"xn_{tcn}_{tt}")
                        t0 = tcn * QCH + tt * P
                        nc.sync.dma_start(out=xn, in_=x[t0 : t0 + P, :])
                        for c in range(8):
                            ps = ps_tp_p.tile([P, P], F32, tag="tp", name=f"tp_{tcn}_{tt}_{c}")
                            nc.tensor.transpose(ps, xn[:, c * P : (c + 1) * P], identity)
                            nc.vector.tensor_copy(
                                xT_t[:, c, tt * P : (tt + 1) * P], ps
                            )
                # qT / kT row groups; each W stationary feeds both chunks
                for r in range(8):
                    pss = [
                        ps_qk_p.tile([P, QCH], F32, tag="qk", name=f"qk_{tcp}_{r}_{i}")
                        for i in range(2)
                    ]
                    for c in range(8):
                        for half in range(2):
                            nc.tensor.matmul(
                                pss[half],
                                lhsT=wa_r[:, c, r * P : (r + 1) * P],
                                rhs=xT_pair[half][:, c, :],
                                start=(c == 0),
                                stop=(c == 7),
                            )
                    for half in range(2):
                        tcn = tcp * 2 + half
                        nc.vector.tensor_copy(
                            qkT_all[:, r, tcn * QCH : (tcn + 1) * QCH], pss[half]
                        )
                # v natural
                for half in range(2):
                    tcn = tcp * 2 + half
                    for tt in range(4):
                        ps = ps_v_p.tile([P, CL], F32, tag="v", name=f"v_{tcn}_{tt}")
                        for c in range(8):
                            nc.tensor.matmul(
                                ps,
                                lhsT=xT_pair[half][:, c, tt * P : (tt + 1) * P],
                                rhs=wa_r[:, c, 2 * CL : 3 * CL],
                                start=(c == 0),
                                stop=(c == 7),
                            )
                        kt = tcn * 4 + tt
                        nc.scalar.copy(
                            v_sb[:, kt, :, 0:D],
                            ps.rearrange("p (h d) -> p h d", h=HL),
                        )

        # ---- phase B: attention --------------------------------------
        # yT_all[:, p, t]: partition 64*(h%2)+d of pair p = normalized y^T.
        # Allocated after phase A closes so it reuses W_attn's SBUF space.
        yT_all, _free_yT = tc.tile([P, 4, T], F32R, name="yT_all")
        # K=128-padded q/k staging for the scores matmul: rows 0:64 hold the
        # current head's qT/kT, rows 64:128 stay zero. A 64-row contraction
        # would put the PE in row-tiled mode, which doesn't count as PE
        # activity for the HAM clock gate -> whole phase stuck at 1.2 GHz.
        # Two fixed buffers ping-pong across heads.
        qk_pads = []
        _pad_frees = []
        for pb in range(2):
            padt, fr = tc.tile([P, 2, T], F32R, name=f"qk_pad{pb}")
            qk_pads.append(padt)
            _pad_frees.append(fr)
        zf, _free_zf = tc.tile([D, T], F32, name="zf")
        nc.vector.memset(zf, 0.0)
        for pb in range(2):
            nc.vector.tensor_copy(qk_pads[pb][D:P, 0, :], zf)
            nc.vector.tensor_copy(qk_pads[pb][D:P, 1, :], zf)
        _free_zf()
        with (
            tc.tile_pool(name="aT", bufs=4) as aT_p,
            tc.tile_pool(name="raw", bufs=2) as raw_p,
            tc.tile_pool(name="recip", bufs=2) as recip_p,
            tc.tile_pool(name="ps_s", bufs=2, space="PSUM") as ps_s_p,
            tc.tile_pool(name="ps_y", bufs=4, space="PSUM") as ps_y_p,
        ):
            def emit_pad_copies(h):
                p_, e = h // 2, h % 2
                qoff = D * e
                pad = qk_pads[h % 2]
                nc.sync.dma_start(
                    out=pad[0:D, 0, :], in_=qkT_all[qoff : qoff + D, p_, :]
                )
                nc.sync.dma_start(
                    out=pad[0:D, 1, :], in_=qkT_all[qoff : qoff + D, 4 + p_, :]
                )

            emit_pad_copies(0)
            for h in range(HL):
                p_, e = h // 2, h % 2
                qoff = D * e
                pad = qk_pads[h % 2]
                qT_h = pad[:, 0, :]
                kT_h = pad[:, 1, :]

                yT_t = [
                    ps_y_p.tile([D + 1, QCH], F32, tag="yT", name=f"yT_{h}_{j}")
                    for j in range(NQC)
                ]
                aT_tiles = [None] * NKT

                def emit_scores(i):
                    span = T - P * i
                    aT_i = aT_p.tile([P, T], F32R, tag="aT")
                    aT_tiles[i] = aT_i
                    # high columns first: PV consumes descending-j, and the
                    # causal mask (cols 0:128) is only needed by the last
                    # (diagonal) PV chunk, so it leaves the critical path
                    subs = list(range(0, span, 1024))
                    for sub in reversed(subs):
                        sw = min(1024, span - sub)
                        ps = ps_s_p.tile([P, 1024], F32, tag="s")
                        for half in range(0, sw, 512):
                            ncol = min(512, sw - half)
                            nc.tensor.matmul(
                                ps[:, half : half + ncol],
                                lhsT=kT_h[:, P * i : P * (i + 1)],
                                rhs=qT_h[:, P * i + sub + half : P * i + sub + half + ncol],
                                start=True,
                                stop=True,
                            )
                        nc.scalar.activation(
                            aT_i[:, sub : sub + sw], ps[:, :sw], EXP, scale=SCALE
                        )
                    # causal mask on the diagonal 128 columns: keep q >= k
                    nc.vector.tensor_mul(aT_i[:, 0:P], aT_i[:, 0:P], tril)

                def emit_pv(i):
                    aT_i = aT_tiles[i]
                    jmin = (P * i) // QCH
                    for j in range(NQC - 1, jmin - 1, -1):
                        qs = QCH * j
                        out_off = max(P * i - qs, 0)
                        rel = max(qs - P * i, 0)
                        ncol = QCH - out_off
                        nc.tensor.matmul(
                            yT_t[j][:, out_off : out_off + ncol],
                            lhsT=v_sb[:, i, h, :],
                            rhs=aT_i[:, rel : rel + ncol],
                            start=(i == 0),
                            stop=(i == 4 * j + 3),
                        )
                    if i % 4 == 3:
                        emit_norm(i // 4)

                def emit_norm(j):
                    raw = raw_p.tile([D + 1, QCH], F32R, tag="raw")
                    nc.vector.tensor_copy(raw, yT_t[j])
                    # broadcast the softmax sums (row 64) across 64 partitions
                    # via a DRAM bounce (DMA with step-0 partition source);
                    # keeps the PE out of it (no mode switch, no PSUM bank).
                    idx = h * NQC + j
                    nc.sync.dma_start(
                        out=l_dram[idx, :], in_=raw[D : D + 1, :].bitcast(F32)
                    )
                    lrow = l_dram[idx, :]
                    bcast_src = bass.AP(
                        tensor=lrow.tensor,
                        offset=lrow.offset,
                        ap=[[0, D]] + [list(pair) for pair in lrow.ap],
                    )
                    bcast = recip_p.tile([D, QCH], F32, tag="bcast")
                    nc.sync.dma_start(out=bcast, in_=bcast_src)
                    recip = recip_p.tile([D, QCH], F32, tag="recip")
                    nc.vector.reciprocal_approx_fast(out=recip, in_=bcast)
                    nc.vector.tensor_mul(
                        yT_all[qoff : qoff + D, p_, QCH * j : QCH * (j + 1)],
                        raw[0:D, :],
                        recip,
                    )

                emit_scores(0)
                emit_scores(1)
                if h + 1 < HL:
                    # stage next head's padded q/k while this head computes
                    emit_pad_copies(h + 1)
                emit_pv(0)
                for i in range(2, NKT):
                    emit_scores(i)
                    emit_pv(i - 1)
                emit_pv(NKT - 1)

        # ---- phase C: output projection ------------------------------
        with (
            tc.tile_pool(name="wpstage", bufs=2) as wpstage_p,
            tc.tile_pool(name="wp_rp", bufs=1) as wp_rp,
            tc.tile_pool(name="outp", bufs=3) as out_p,
            tc.tile_pool(name="ps_o", bufs=4, space="PSUM") as ps_o_p,
        ):
            wp_r = wp_rp.tile([P, 4, C], F32R)
            wp_t = wp.rearrange("(c p) n -> p c n", p=P)  # [128, 4, 1024]
            for cc in range(4):
                wst = wpstage_p.tile([P, C], F32, tag="wpst")
                nc.sync.dma_start(out=wst, in_=wp_t[:, cc, :])
                nc.scalar.copy(wp_r[:, cc, :], wst)

            for t in range(NKT):
                out_t = out_p.tile([P, C], F32, tag="out")
                ps_h = [
                    ps_o_p.tile([P, 512], F32, tag="o", name=f"ps_o_{t}_{hf}")
                    for hf in range(2)
                ]
                for p2 in range(4):
                    for half in range(2):
                        nc.tensor.matmul(
                            ps_h[half],
                            lhsT=yT_all[:, p2, P * t : P * (t + 1)],
                            rhs=wp_r[:, p2, 512 * half : 512 * (half + 1)],
                            start=(p2 == 0),
                            stop=(p2 == 3),
                        )
                for half in range(2):
                    nc.vector.tensor_copy(
                        out_t[:, 512 * half : 512 * (half + 1)], ps_h[half]
                    )
                nc.sync.dma_start(out=out[P * t : P * (t + 1), :], in_=out_t)

    nc.finalize()
    return nc


_NC_CACHE = None


def kernel(x, W_attn, W_proj):
    global _NC_CACHE
    if _NC_CACHE is None:
        _NC_CACHE = build_kernel()
    nc = _NC_CACHE

    B = x.shape[0]
    H_TOTAL = 16
    x = np.asarray(x, dtype=np.float32)
    W_attn = np.asarray(W_attn, dtype=np.float32)
    W_proj = np.asarray(W_proj, dtype=np.float32)

    in_maps = []
    for core in range(8):
        b, g = core // 2, core % 2
        cols = slice(g * CL, (g + 1) * CL)
        wa_local = np.ascontiguousarray(
            np.concatenate(
                [
                    W_attn[:, 0 * C :][:, cols],
                    W_attn[:, 1 * C :][:, cols],
                    W_attn[:, 2 * C :][:, cols],
                ],
                axis=1,
            )
        )
        wp_local = np.ascontiguousarray(W_proj[g * CL : (g + 1) * CL, :])
        in_maps.append(
            {
                "x": np.ascontiguousarray(x[b]),
                "wa": wa_local,
                "wp": wp_local,
            }
        )

    res = run_bass_kernel_spmd(nc, in_maps, core_ids=list(range(8)))
    outs = [r["out"] for r in res.results]
    full = np.stack([outs[2 * b] + outs[2 * b + 1] for b in range(B)], axis=0)
    return full
